# revision 11
# baseline (speedup 1.0000x reference)
"""AthenaSA sliding-window attention layer on 8 TRN2 NeuronCores.

Sharding: sequence-parallel. 8 cores = 2 batches x 4 sequence chunks of 1024
tokens. Each core recomputes k/v for a 512-token halo (zero-padded for the
first chunk), so there are NO collectives — the kernel is embarrassingly
parallel and each core runs an identical Bass graph on different data.

Per-core pipeline (all matmuls bf16, accumulation f32):
  embT [E,1536] (host-pre-transposed) -> RMSNorm stats via ones-matmul
  partition reduction -> normedT bf16 -> QK projections in transposed layout
  (q^T, k^T = [dk, tokens]) + RoPE (partition-shifted reads) -> V projection
  in natural layout -> banded sliding-window attention with transposed
  scores (scores^T = k-chunk @ q^T), multiplicative data-driven masks after
  exp, denominators via all-ones stationary matmul (broadcast across
  partitions) -> out-projection back to natural layout + residual.
"""
import math
import os
import sys

sys.path.insert(0, "/opt/trn_rl_repo")

import numpy as np
import ml_dtypes

import concourse.bass as bass
import concourse.bacc as bacc
import concourse.mybir as mybir
from concourse import tile
from concourse import bass_utils
from contextlib import ExitStack

BF16 = ml_dtypes.bfloat16

B, S, E = 2, 4096, 2048
H, HKV, DK, DV = 16, 4, 128, 128
WIN = 512
EPS = 1e-5
TOWN, TALL, HALO = 1024, 1536, 512
NE = E // 128            # 16 e-tiles
NB = 2                   # window blocks per core
NQC = 4                  # query tiles of 128 per block
NCH = 5                  # key chunks of 128 per 640-window
GQ = H // HKV

f32 = mybir.dt.float32
bf = mybir.dt.bfloat16
AF = mybir.ActivationFunctionType


def build(tc, d):
    nc = tc.nc

    with ExitStack() as stage_all:
        stage_all.enter_context(
            nc.allow_low_precision(reason="bf16 compute path by design"))
        const_pool = stage_all.enter_context(tc.tile_pool(name="const", bufs=1))
        ones = const_pool.tile([128, 128], bf)
        nc.gpsimd.memset(ones[:], 1.0)
        epsb = const_pool.tile([128, 1], f32)
        nc.gpsimd.memset(epsb[:], EPS)

        # manually-scoped pools; LIFO open/close order:
        # acat -> kT -> v -> qT -> normedT
        acat_cm = tc.tile_pool(name="acat", bufs=H)           # ..D
        acat_pool = acat_cm.__enter__()
        kT_cm = tc.tile_pool(name="kT", bufs=HKV)             # ..C
        kT_pool = kT_cm.__enter__()
        v_cm = tc.tile_pool(name="v", bufs=12)                # ..C
        v_pool = v_cm.__enter__()
        qT_cm = tc.tile_pool(name="qT", bufs=H)               # ..C
        qT_pool = qT_cm.__enter__()
        normedT_cm = tc.tile_pool(name="normedT", bufs=NE)    # ..B2
        normedT_pool = normedT_cm.__enter__()

        # ---------------- Stage A: RMSNorm stats + normalize ----------------
        normedT = []
        with ExitStack() as sa:
            ef_pool = sa.enter_context(tc.tile_pool(name="embTf32", bufs=3))
            sq_pool = sa.enter_context(tc.tile_pool(name="sq", bufs=3))
            r_pool = sa.enter_context(tc.tile_pool(name="rms", bufs=1))
            ssq_psum = sa.enter_context(
                tc.tile_pool(name="ssq_ps", bufs=1, space="PSUM"))

            ssq = ssq_psum.tile([128, TALL], f32)  # 3 banks
            for e in range(NE):
                ef = ef_pool.tile([128, TALL], f32)
                nc.sync.dma_start(ef[:], d["embT"][e * 128:(e + 1) * 128, :])
                nt = normedT_pool.tile([128, TALL], bf, name="nt")
                nc.vector.tensor_copy(nt[:], ef[:])          # cast f32->bf16
                normedT.append(nt)
                sq = sq_pool.tile([128, TALL], bf)
                nc.vector.tensor_mul(sq[:], nt[:], nt[:])
                for j in range(3):
                    nc.tensor.matmul(
                        ssq[:, j * 512:(j + 1) * 512], ones[:],
                        sq[:, j * 512:(j + 1) * 512],
                        start=(e == 0), stop=(e == NE - 1))
            # rms = sqrt(ssq/E + eps); rb = 1/rms  (all rows identical)
            s_sb = r_pool.tile([128, TALL], f32)
            nc.scalar.activation(s_sb[:], ssq[:], AF.Sqrt,
                                 bias=epsb[:], scale=1.0 / E)
            rb = r_pool.tile([128, TALL], bf)
            nc.vector.reciprocal(rb[:], s_sb[:])             # f32 in, bf16 out
            for e in range(NE):
                nc.vector.tensor_mul(normedT[e][:], normedT[e][:], rb[:])

        # ---------------- Stage B1: K^T + V projections ----------------
        kT = []   # per kv head: [128(dk), TALL] bf16, rope'd
        vtiles = []  # 12 tiles [128(tok), 512(kv*dv)] bf16
        with ExitStack() as sb1:
            wk_pool = sb1.enter_context(tc.tile_pool(name="wk", bufs=NE))
            wv_pool = sb1.enter_context(tc.tile_pool(name="wv", bufs=NE))
            cs_pool = sb1.enter_context(tc.tile_pool(name="cosk", bufs=1))
            tmp_pool = sb1.enter_context(tc.tile_pool(name="ropetmp", bufs=2))
            kps_pool = sb1.enter_context(
                tc.tile_pool(name="k_ps", bufs=2, space="PSUM"))
            vps_pool = sb1.enter_context(
                tc.tile_pool(name="v_ps", bufs=2, space="PSUM"))

            wkt = []
            wvt = []
            for e in range(NE):
                wk_t = wk_pool.tile([128, HKV * DK], bf, name="wk_t")
                nc.sync.dma_start(wk_t[:], d["wk"][e * 128:(e + 1) * 128, :])
                wkt.append(wk_t)
                wv_t = wv_pool.tile([128, HKV * DV], bf, name="wv_t")
                nc.sync.dma_start(wv_t[:], d["wv"][e * 128:(e + 1) * 128, :])
                wvt.append(wv_t)
            cosk = cs_pool.tile([128, TALL], bf)
            sink = cs_pool.tile([128, TALL], bf)
            nc.sync.dma_start(cosk[:], d["coskT"][:, :])
            nc.sync.dma_start(sink[:], d["sinkT"][:, :])

            for hk in range(HKV):
                kps = kps_pool.tile([128, TALL], f32)  # 3 banks
                for e in range(NE):
                    for j in range(3):
                        nc.tensor.matmul(
                            kps[:, j * 512:(j + 1) * 512],
                            wkt[e][:, hk * DK:(hk + 1) * DK],
                            normedT[e][:, j * 512:(j + 1) * 512],
                            start=(e == 0), stop=(e == NE - 1))
                # rope: out = cos*kraw + sin*swap(kraw); swap rotates the two
                # 64-partition halves
                kraw = tmp_pool.tile([128, TALL], bf)
                nc.scalar.copy(kraw[:], kps[:])
                ksw = tmp_pool.tile([128, TALL], bf)
                nc.sync.dma_start(ksw[0:64, :], kraw[64:128, :])
                nc.sync.dma_start(ksw[64:128, :], kraw[0:64, :])
                t1 = tmp_pool.tile([128, TALL], bf)
                nc.vector.tensor_mul(t1[:], kraw[:], cosk[:])
                ko = kT_pool.tile([128, TALL], bf, name="ko")
                nc.vector.tensor_mul(ko[:], ksw[:], sink[:])
                nc.vector.tensor_add(ko[:], ko[:], t1[:])
                kT.append(ko)

            for t in range(12):
                vps = vps_pool.tile([128, HKV * DV], f32)  # 1 bank
                for e in range(NE):
                    nc.tensor.matmul(
                        vps[:], normedT[e][:, t * 128:(t + 1) * 128],
                        wvt[e][:], start=(e == 0), stop=(e == NE - 1))
                vt = v_pool.tile([128, HKV * DV], bf, name="vt")
                nc.scalar.copy(vt[:], vps[:])
                vtiles.append(vt)

        # ---------------- Stage B2: Q^T projection ----------------
        # wq comes in host-permuted per-head layout [H, E, DK] so per-head
        # weight tiles DMA contiguously and only ~3 heads stay resident.
        qT = []
        with ExitStack() as sb2:
            wq_pool = sb2.enter_context(tc.tile_pool(name="wq", bufs=3 * NE))
            csq_pool = sb2.enter_context(tc.tile_pool(name="cosq", bufs=1))
            tmpq_pool = sb2.enter_context(tc.tile_pool(name="ropetmpq", bufs=2))
            qps_pool = sb2.enter_context(
                tc.tile_pool(name="q_ps", bufs=2, space="PSUM"))

            cosq = csq_pool.tile([128, TOWN], bf)
            sinq = csq_pool.tile([128, TOWN], bf)
            nc.sync.dma_start(cosq[:], d["cosqT"][:, :])
            nc.sync.dma_start(sinq[:], d["sinqT"][:, :])

            for h in range(H):
                wqh = []
                for e in range(NE):
                    wq_t = wq_pool.tile([128, DK], bf, name="wq_t")
                    nc.sync.dma_start(wq_t[:],
                                      d["wq"][h, e * 128:(e + 1) * 128, :])
                    wqh.append(wq_t)
                qps = qps_pool.tile([128, TOWN], f32)  # 2 banks
                for e in range(NE):
                    for j in range(2):
                        nc.tensor.matmul(
                            qps[:, j * 512:(j + 1) * 512],
                            wqh[e][:],
                            normedT[e][:, HALO + j * 512:HALO + (j + 1) * 512],
                            start=(e == 0), stop=(e == NE - 1))
                qraw = tmpq_pool.tile([128, TOWN], bf)
                nc.scalar.copy(qraw[:], qps[:])
                qsw = tmpq_pool.tile([128, TOWN], bf)
                nc.sync.dma_start(qsw[0:64, :], qraw[64:128, :])
                nc.sync.dma_start(qsw[64:128, :], qraw[0:64, :])
                t1 = tmpq_pool.tile([128, TOWN], bf, name="t1q")
                nc.vector.tensor_mul(t1[:], qraw[:], cosq[:])
                qo = qT_pool.tile([128, TOWN], bf, name="qo")
                nc.vector.tensor_mul(qo[:], qsw[:], sinq[:])
                nc.vector.tensor_add(qo[:], qo[:], t1[:])
                qT.append(qo)
        normedT_cm.__exit__(None, None, None)

        # ---------------- Stage C: attention ----------------
        acat = []
        for h in range(H):
            acat.append(acat_pool.tile([128, TOWN], bf, name="acat"))

        with ExitStack() as sc_stage:
            mask_pool = sc_stage.enter_context(
                tc.tile_pool(name="mask", bufs=NB * NQC))
            probs_pool = sc_stage.enter_context(tc.tile_pool(name="probs", bufs=3))
            rec_pool = sc_stage.enter_context(tc.tile_pool(name="rec", bufs=3))
            scps_pool = sc_stage.enter_context(
                tc.tile_pool(name="sc_ps", bufs=2, space="PSUM"))
            dnps_pool = sc_stage.enter_context(
                tc.tile_pool(name="dn_ps", bufs=2, space="PSUM"))
            otps_pool = sc_stage.enter_context(
                tc.tile_pool(name="ot_ps", bufs=2, space="PSUM"))

            masks = {}
            for blk in range(NB):
                for qc in range(NQC):
                    m = mask_pool.tile([128, NCH, 128], bf, name="m")
                    nc.sync.dma_start(m[:], d["maskT"][blk, qc])
                    masks[(blk, qc)] = m

            for h in range(H):
                kv = h // GQ
                for blk in range(NB):
                    for qc in range(NQC):
                        w0 = 512 * blk + 128 * qc     # key window start (local)
                        qs = 512 * blk + 128 * qc     # query tile start (own)
                        scp = scps_pool.tile([128, NCH, 128], f32)
                        for ch in range(NCH):
                            nc.tensor.matmul(
                                scp[:, ch, :],
                                kT[kv][:, w0 + ch * 128:w0 + (ch + 1) * 128],
                                qT[h][:, qs:qs + 128],
                                start=True, stop=True)
                        probs = probs_pool.tile([128, NCH, 128], bf)
                        nc.scalar.activation(probs[:], scp[:], AF.Exp)
                        nc.vector.tensor_mul(probs[:], probs[:],
                                             masks[(blk, qc)][:])
                        dn = dnps_pool.tile([128, 128], f32)
                        for ch in range(NCH):
                            nc.tensor.matmul(dn[:], ones[:], probs[:, ch, :],
                                             start=(ch == 0), stop=(ch == NCH - 1))
                        rec = rec_pool.tile([128, 128], f32)
                        nc.vector.reciprocal(rec[:], dn[:])
                        otp = otps_pool.tile([128, 128], f32)
                        for ch in range(NCH):
                            vt = vtiles[4 * blk + qc + ch]
                            nc.tensor.matmul(
                                otp[:], vt[:, kv * DV:(kv + 1) * DV],
                                probs[:, ch, :],
                                start=(ch == 0), stop=(ch == NCH - 1))
                        nc.vector.tensor_mul(acat[h][:, qs:qs + 128],
                                             otp[:], rec[:])
        qT_cm.__exit__(None, None, None)
        v_cm.__exit__(None, None, None)
        kT_cm.__exit__(None, None, None)

        # ---------------- Stage D: out projection + residual ----------
        with ExitStack() as sd:
            wo_pool = sd.enter_context(tc.tile_pool(name="wo", bufs=NE))
            emb_pool = sd.enter_context(tc.tile_pool(name="embown", bufs=3))
            out_pool = sd.enter_context(tc.tile_pool(name="outsb", bufs=3))
            ops_pool = sd.enter_context(
                tc.tile_pool(name="op_ps", bufs=4, space="PSUM"))

            wot = []
            for e in range(NE):
                wo_t = wo_pool.tile([128, E], bf, name="wo_t")
                nc.sync.dma_start(wo_t[:], d["wo"][e * 128:(e + 1) * 128, :])
                wot.append(wo_t)

            for t in range(8):
                emb_t = emb_pool.tile([128, E], f32)
                nc.sync.dma_start(emb_t[:],
                                  d["emb_own"][t * 128:(t + 1) * 128, :])
                out_sb = out_pool.tile([128, E], f32)
                for j in range(4):
                    op = ops_pool.tile([128, 512], f32)
                    for kh in range(NE):
                        nc.tensor.matmul(
                            op[:], acat[kh][:, t * 128:(t + 1) * 128],
                            wot[kh][:, j * 512:(j + 1) * 512],
                            start=(kh == 0), stop=(kh == NE - 1))
                    nc.vector.tensor_add(out_sb[:, j * 512:(j + 1) * 512],
                                         op[:], emb_t[:, j * 512:(j + 1) * 512])
                nc.sync.dma_start(d["out"][t * 128:(t + 1) * 128, :],
                                  out_sb[:])
        acat_cm.__exit__(None, None, None)


_CACHED_NC = None


def build_graph():
    global _CACHED_NC
    if _CACHED_NC is not None:
        return _CACHED_NC
    nc = bacc.Bacc("TRN2", target_bir_lowering=False, debug=False,
                   enable_asserts=False, num_devices=8)
    d = {}
    d["embT"] = nc.dram_tensor("embT", [E, TALL], f32, kind="ExternalInput").ap()
    d["emb_own"] = nc.dram_tensor("emb_own", [TOWN, E], f32,
                                  kind="ExternalInput").ap()
    d["wq"] = nc.dram_tensor("wq", [H, E, DK], bf, kind="ExternalInput").ap()
    d["wk"] = nc.dram_tensor("wk", [E, HKV * DK], bf, kind="ExternalInput").ap()
    d["wv"] = nc.dram_tensor("wv", [E, HKV * DV], bf, kind="ExternalInput").ap()
    d["wo"] = nc.dram_tensor("wo", [H * DV, E], bf, kind="ExternalInput").ap()
    d["cosqT"] = nc.dram_tensor("cosqT", [DK, TOWN], bf, kind="ExternalInput").ap()
    d["sinqT"] = nc.dram_tensor("sinqT", [DK, TOWN], bf, kind="ExternalInput").ap()
    d["coskT"] = nc.dram_tensor("coskT", [DK, TALL], bf, kind="ExternalInput").ap()
    d["sinkT"] = nc.dram_tensor("sinkT", [DK, TALL], bf, kind="ExternalInput").ap()
    d["maskT"] = nc.dram_tensor("maskT", [NB, NQC, 128, NCH, 128], bf,
                                kind="ExternalInput").ap()
    d["out"] = nc.dram_tensor("out", [TOWN, E], f32, kind="ExternalOutput").ap()

    with tile.TileContext(nc, trace_sim=False) as tc:
        build(tc, d)
    nc.compile()
    _CACHED_NC = nc
    return nc


def make_in_maps(embeddings, cos_buffer, sin_buffer, wq, wk, wv, wo):
    embeddings = np.asarray(embeddings, dtype=np.float32)
    cos_buffer = np.asarray(cos_buffer, dtype=np.float32)
    sin_buffer = np.asarray(sin_buffer, dtype=np.float32)
    wq_s = (np.asarray(wq, np.float32) / math.sqrt(DK)).astype(BF16)
    wq_s = np.ascontiguousarray(wq_s.reshape(E, H, DK).transpose(1, 0, 2))
    wk_b = np.asarray(wk, np.float32).astype(BF16)
    wv_b = np.asarray(wv, np.float32).astype(BF16)
    wo_b = np.asarray(wo, np.float32).astype(BF16)

    in_maps = []
    for core in range(8):
        b, c = divmod(core, 4)
        tok0 = 1024 * c
        if c == 0:
            pad = np.zeros((HALO, E), np.float32)
            seg = np.concatenate([pad, embeddings[b, :TOWN]], axis=0)
            padc = np.zeros((HALO, DK), np.float32)
            ck = np.concatenate([padc, cos_buffer[1, 0, :TOWN]], axis=0)
            sk = np.concatenate([padc, sin_buffer[1, 0, :TOWN]], axis=0)
        else:
            seg = embeddings[b, tok0 - HALO:tok0 + TOWN]
            ck = cos_buffer[1, 0, tok0 - HALO:tok0 + TOWN]
            sk = sin_buffer[1, 0, tok0 - HALO:tok0 + TOWN]

        # masks [NB, NQC, 128(kk), NCH, 128(qq)] {0,1}
        mask = np.zeros((NB, NQC, 128, NCH, 128), np.float32)
        qq = np.arange(128)
        kk = np.arange(128)
        for blk in range(NB):
            for qc in range(NQC):
                qpos = tok0 + 512 * blk + 128 * qc + qq
                for ch in range(NCH):
                    kpos = tok0 - 512 + 512 * blk + 128 * qc + 128 * ch + kk
                    m = ((kpos[:, None] > qpos[None, :] - WIN)
                         & (kpos[:, None] <= qpos[None, :])
                         & (kpos[:, None] >= 0))
                    mask[blk, qc, :, ch, :] = m

        in_maps.append({
            "embT": np.ascontiguousarray(seg.T),
            "emb_own": np.ascontiguousarray(embeddings[b, tok0:tok0 + TOWN]),
            "wq": wq_s, "wk": wk_b, "wv": wv_b, "wo": wo_b,
            "cosqT": np.ascontiguousarray(
                cos_buffer[0, 0, tok0:tok0 + TOWN].T).astype(BF16),
            "sinqT": np.ascontiguousarray(
                sin_buffer[0, 0, tok0:tok0 + TOWN].T).astype(BF16),
            "coskT": np.ascontiguousarray(ck.T).astype(BF16),
            "sinkT": np.ascontiguousarray(sk.T).astype(BF16),
            "maskT": mask.astype(BF16),
        })
    return in_maps


def _install_ntff_hook():
    """Recreate the missing antenv.axon_hooks registry so
    run_bass_kernel_spmd(trace=True) can capture an NTFF profile."""
    import types
    if "antenv.axon_hooks" not in sys.modules:
        m = types.ModuleType("antenv.axon_hooks")
        m._hook = None
        m.set_axon_ntff_profile_hook = lambda h: setattr(m, "_hook", h)
        m.get_axon_ntff_profile_hook = lambda: m._hook
        sys.modules["antenv.axon_hooks"] = m
        try:
            import antenv
            antenv.axon_hooks = m
        except ImportError:
            pass
    try:
        from trn_agent_boot.trn_boot import _ntff_profile_via_ctypes
        hook = _ntff_profile_via_ctypes("/opt/axon/libaxon_pjrt.so")
        sys.modules["antenv.axon_hooks"].set_axon_ntff_profile_hook(hook)
    except Exception as exc:  # degrade to no tracing
        print(f"ntff hook install failed: {exc}", file=sys.stderr)


def kernel(embeddings, cos_buffer, sin_buffer, wq, wk, wv, wo, window_size,
           trace=False):
    assert int(window_size) == WIN
    if trace:
        _install_ntff_hook()
    nc = build_graph()
    in_maps = make_in_maps(embeddings, cos_buffer, sin_buffer, wq, wk, wv, wo)
    res = bass_utils.run_bass_kernel_spmd(
        nc, in_maps, core_ids=list(range(8)), trace=trace)
    out = np.zeros((B, S, E), np.float32)
    for core in range(8):
        b, c = divmod(core, 4)
        out[b, 1024 * c:1024 * (c + 1)] = res.results[core]["out"]
    if trace:
        kernel.last_exec_time_ns = res.exec_time_ns
    return out


kernel.last_exec_time_ns = None


# revision 13
# speedup vs baseline: 1.2826x; 1.2826x over previous
"""AthenaSA sliding-window attention layer on 8 TRN2 NeuronCores.

Sharding: sequence-parallel. 8 cores = 2 batches x 4 sequence chunks of 1024
tokens. Each core recomputes k/v for a 512-token halo (zero-padded for the
first chunk), so there are NO collectives — the kernel is embarrassingly
parallel and each core runs an identical Bass graph on different data.

Per-core pipeline (all matmuls bf16, accumulation f32):
  embT [E,1536] (host-pre-transposed) -> RMSNorm stats via ones-matmul
  partition reduction -> normedT bf16 -> QK projections in transposed layout
  (q^T, k^T = [dk, tokens]) + RoPE (partition-shifted reads) -> V projection
  in natural layout -> banded sliding-window attention with transposed
  scores (scores^T = k-chunk @ q^T), multiplicative data-driven masks after
  exp, denominators via all-ones stationary matmul (broadcast across
  partitions) -> out-projection back to natural layout + residual.
"""
import math
import os
import sys

sys.path.insert(0, "/opt/trn_rl_repo")

import numpy as np
import ml_dtypes

import concourse.bass as bass
import concourse.bacc as bacc
import concourse.mybir as mybir
from concourse import tile
from concourse import bass_utils
from contextlib import ExitStack

BF16 = ml_dtypes.bfloat16

B, S, E = 2, 4096, 2048
H, HKV, DK, DV = 16, 4, 128, 128
WIN = 512
EPS = 1e-5
TOWN, TALL, HALO = 1024, 1536, 512
NE = E // 128            # 16 e-tiles
NB = 2                   # window blocks per core
NQC = 4                  # query tiles of 128 per block
NCH = 5                  # key chunks of 128 per 640-window
GQ = H // HKV

f32 = mybir.dt.float32
bf = mybir.dt.bfloat16
AF = mybir.ActivationFunctionType


def build(tc, d):
    nc = tc.nc

    with ExitStack() as stage_all:
        stage_all.enter_context(
            nc.allow_low_precision(reason="bf16 compute path by design"))
        const_pool = stage_all.enter_context(tc.tile_pool(name="const", bufs=1))
        ones = const_pool.tile([128, 128], bf)
        nc.gpsimd.memset(ones[:], 1.0)
        epsb = const_pool.tile([128, 1], f32)
        nc.gpsimd.memset(epsb[:], EPS)

        # manually-scoped pools; LIFO open/close order:
        # acat -> kT -> v -> qT -> normedT
        acat_cm = tc.tile_pool(name="acat", bufs=H)           # ..D
        acat_pool = acat_cm.__enter__()
        kT_cm = tc.tile_pool(name="kT", bufs=HKV)             # ..C
        kT_pool = kT_cm.__enter__()
        v_cm = tc.tile_pool(name="v", bufs=12)                # ..C
        v_pool = v_cm.__enter__()
        qT_cm = tc.tile_pool(name="qT", bufs=H)               # ..C
        qT_pool = qT_cm.__enter__()
        normedT_cm = tc.tile_pool(name="normedT", bufs=NE)    # ..B2
        normedT_pool = normedT_cm.__enter__()

        # ---------------- Stage A: RMSNorm stats + normalize ----------------
        normedT = []
        with ExitStack() as sa:
            ef_pool = sa.enter_context(tc.tile_pool(name="embTf32", bufs=3))
            sq_pool = sa.enter_context(tc.tile_pool(name="sq", bufs=3))
            r_pool = sa.enter_context(tc.tile_pool(name="rms", bufs=1))
            ssq_psum = sa.enter_context(
                tc.tile_pool(name="ssq_ps", bufs=1, space="PSUM"))

            ssq = ssq_psum.tile([128, TALL], f32)  # 3 banks
            for e in range(NE):
                ef = ef_pool.tile([128, TALL], f32)
                nc.sync.dma_start(ef[:], d["embT"][e * 128:(e + 1) * 128, :])
                nt = normedT_pool.tile([128, TALL], bf, name="nt")
                nc.vector.tensor_copy(nt[:], ef[:])          # cast f32->bf16
                normedT.append(nt)
                sq = sq_pool.tile([128, TALL], bf)
                nc.vector.tensor_mul(sq[:], nt[:], nt[:])
                for j in range(3):
                    nc.tensor.matmul(
                        ssq[:, j * 512:(j + 1) * 512], ones[:],
                        sq[:, j * 512:(j + 1) * 512],
                        start=(e == 0), stop=(e == NE - 1))
            # rms = sqrt(ssq/E + eps); rb = 1/rms  (all rows identical)
            s_sb = r_pool.tile([128, TALL], f32)
            nc.scalar.activation(s_sb[:], ssq[:], AF.Sqrt,
                                 bias=epsb[:], scale=1.0 / E)
            rb = r_pool.tile([128, TALL], bf)
            nc.vector.reciprocal(rb[:], s_sb[:])             # f32 in, bf16 out
            for e in range(NE):
                nc.vector.tensor_mul(normedT[e][:], normedT[e][:], rb[:])

        # ---------------- Stage B1: K^T + V projections ----------------
        kT = []   # per kv head: [128(dk), TALL] bf16, rope'd
        vtiles = []  # 12 tiles [128(tok), 512(kv*dv)] bf16
        with ExitStack() as sb1:
            wk_pool = sb1.enter_context(tc.tile_pool(name="wk", bufs=NE))
            wv_pool = sb1.enter_context(tc.tile_pool(name="wv", bufs=NE))
            cs_pool = sb1.enter_context(tc.tile_pool(name="cosk", bufs=1))
            tmp_pool = sb1.enter_context(tc.tile_pool(name="ropetmp", bufs=3))
            kps_pool = sb1.enter_context(
                tc.tile_pool(name="k_ps", bufs=2, space="PSUM"))
            vps_pool = sb1.enter_context(
                tc.tile_pool(name="v_ps", bufs=2, space="PSUM"))

            wkt = []
            wvt = []
            for e in range(NE):
                wk_t = wk_pool.tile([128, HKV * DK], bf, name="wk_t")
                nc.sync.dma_start(wk_t[:], d["wk"][e * 128:(e + 1) * 128, :])
                wkt.append(wk_t)
                wv_t = wv_pool.tile([128, HKV * DV], bf, name="wv_t")
                nc.sync.dma_start(wv_t[:], d["wv"][e * 128:(e + 1) * 128, :])
                wvt.append(wv_t)
            cosk = cs_pool.tile([128, TALL], bf)
            sink = cs_pool.tile([128, TALL], bf)
            nc.sync.dma_start(cosk[:], d["coskT"][:, :])
            nc.sync.dma_start(sink[:], d["sinkT"][:, :])

            for hk in range(HKV):
                kps = kps_pool.tile([128, TALL], f32)  # 3 banks
                for e in range(NE):
                    for j in range(3):
                        nc.tensor.matmul(
                            kps[:, j * 512:(j + 1) * 512],
                            wkt[e][:, hk * DK:(hk + 1) * DK],
                            normedT[e][:, j * 512:(j + 1) * 512],
                            start=(e == 0), stop=(e == NE - 1))
                # rope: out = cos*kraw + sin*swap(kraw); swap rotates the two
                # 64-partition halves
                kraw = tmp_pool.tile([128, TALL], bf)
                nc.scalar.copy(kraw[:], kps[:])
                ksw = tmp_pool.tile([128, TALL], bf)
                nc.sync.dma_start(ksw[0:64, :], kraw[64:128, :])
                nc.sync.dma_start(ksw[64:128, :], kraw[0:64, :])
                t1 = tmp_pool.tile([128, TALL], bf)
                nc.vector.tensor_mul(t1[:], kraw[:], cosk[:])
                ko = kT_pool.tile([128, TALL], bf, name="ko")
                nc.vector.tensor_mul(ko[:], ksw[:], sink[:])
                nc.vector.tensor_add(ko[:], ko[:], t1[:])
                kT.append(ko)

            for t in range(12):
                vps = vps_pool.tile([128, HKV * DV], f32)  # 1 bank
                for e in range(NE):
                    nc.tensor.matmul(
                        vps[:], normedT[e][:, t * 128:(t + 1) * 128],
                        wvt[e][:], start=(e == 0), stop=(e == NE - 1))
                vt = v_pool.tile([128, HKV * DV], bf, name="vt")
                nc.scalar.copy(vt[:], vps[:])
                vtiles.append(vt)

        # ---------------- Stage B2: Q^T projection ----------------
        # wq comes in host-permuted per-head layout [H, E, DK] so per-head
        # weight tiles DMA contiguously and only ~3 heads stay resident.
        qT = []
        with ExitStack() as sb2:
            wq_pool = sb2.enter_context(tc.tile_pool(name="wq", bufs=3 * NE))
            csq_pool = sb2.enter_context(tc.tile_pool(name="cosq", bufs=1))
            tmpq_pool = sb2.enter_context(tc.tile_pool(name="ropetmpq", bufs=3))
            qps_pool = sb2.enter_context(
                tc.tile_pool(name="q_ps", bufs=3, space="PSUM"))

            cosq = csq_pool.tile([128, TOWN], bf)
            sinq = csq_pool.tile([128, TOWN], bf)
            nc.sync.dma_start(cosq[:], d["cosqT"][:, :])
            nc.sync.dma_start(sinq[:], d["sinqT"][:, :])

            for h in range(H):
                wqh = []
                for e in range(NE):
                    wq_t = wq_pool.tile([128, DK], bf, name="wq_t")
                    nc.sync.dma_start(wq_t[:],
                                      d["wq"][h, e * 128:(e + 1) * 128, :])
                    wqh.append(wq_t)
                qps = qps_pool.tile([128, TOWN], f32)  # 2 banks
                for e in range(NE):
                    for j in range(2):
                        nc.tensor.matmul(
                            qps[:, j * 512:(j + 1) * 512],
                            wqh[e][:],
                            normedT[e][:, HALO + j * 512:HALO + (j + 1) * 512],
                            start=(e == 0), stop=(e == NE - 1))
                qraw = tmpq_pool.tile([128, TOWN], bf)
                nc.scalar.copy(qraw[:], qps[:])
                qsw = tmpq_pool.tile([128, TOWN], bf)
                nc.sync.dma_start(qsw[0:64, :], qraw[64:128, :])
                nc.sync.dma_start(qsw[64:128, :], qraw[0:64, :])
                t1 = tmpq_pool.tile([128, TOWN], bf, name="t1q")
                nc.vector.tensor_mul(t1[:], qraw[:], cosq[:])
                qo = qT_pool.tile([128, TOWN], bf, name="qo")
                nc.vector.tensor_mul(qo[:], qsw[:], sinq[:])
                nc.vector.tensor_add(qo[:], qo[:], t1[:])
                qT.append(qo)
        normedT_cm.__exit__(None, None, None)

        # ---------------- Stage C: attention ----------------
        acat = []
        for h in range(H):
            acat.append(acat_pool.tile([128, TOWN], bf, name="acat"))

        with ExitStack() as sc_stage:
            mask_pool = sc_stage.enter_context(
                tc.tile_pool(name="mask", bufs=NB * NQC))
            probs_pool = sc_stage.enter_context(tc.tile_pool(name="probs", bufs=3))
            rec_pool = sc_stage.enter_context(tc.tile_pool(name="rec", bufs=3))
            scps_pool = sc_stage.enter_context(
                tc.tile_pool(name="sc_ps", bufs=2, space="PSUM"))
            dnps_pool = sc_stage.enter_context(
                tc.tile_pool(name="dn_ps", bufs=2, space="PSUM"))
            otps_pool = sc_stage.enter_context(
                tc.tile_pool(name="ot_ps", bufs=2, space="PSUM"))

            masks = {}
            for blk in range(NB):
                for qc in range(NQC):
                    m = mask_pool.tile([128, NCH * 128], bf, name="m")
                    nc.sync.dma_start(
                        m[:], d["maskT"][blk, qc].rearrange("k c q -> k (c q)"))
                    masks[(blk, qc)] = m

            for h in range(H):
                kv = h // GQ
                for blk in range(NB):
                    for qc in range(NQC):
                        w0 = 512 * blk + 128 * qc     # key window start (local)
                        qs = 512 * blk + 128 * qc     # query tile start (own)
                        scp = scps_pool.tile([128, NCH * 128], f32)
                        for ch in range(NCH):
                            nc.tensor.matmul(
                                scp[:, ch * 128:(ch + 1) * 128],
                                kT[kv][:, w0 + ch * 128:w0 + (ch + 1) * 128],
                                qT[h][:, qs:qs + 128],
                                start=True, stop=True)
                        probs = probs_pool.tile([128, NCH * 128], bf)
                        nc.scalar.activation(probs[:], scp[:], AF.Exp)
                        nc.vector.tensor_mul(probs[:], probs[:],
                                             masks[(blk, qc)][:])
                        dn = dnps_pool.tile([128, 128], f32)
                        for ch in range(NCH):
                            nc.tensor.matmul(
                                dn[:], ones[:],
                                probs[:, ch * 128:(ch + 1) * 128],
                                start=(ch == 0), stop=(ch == NCH - 1))
                        rec = rec_pool.tile([128, 128], f32)
                        nc.vector.reciprocal_approx_fast(rec[:], dn[:])
                        otp = otps_pool.tile([128, 128], f32)
                        for ch in range(NCH):
                            vt = vtiles[4 * blk + qc + ch]
                            nc.tensor.matmul(
                                otp[:], vt[:, kv * DV:(kv + 1) * DV],
                                probs[:, ch * 128:(ch + 1) * 128],
                                start=(ch == 0), stop=(ch == NCH - 1))
                        nc.vector.tensor_mul(acat[h][:, qs:qs + 128],
                                             otp[:], rec[:])
        qT_cm.__exit__(None, None, None)
        v_cm.__exit__(None, None, None)
        kT_cm.__exit__(None, None, None)

        # ---------------- Stage D: out projection + residual ----------
        with ExitStack() as sd:
            wo_pool = sd.enter_context(tc.tile_pool(name="wo", bufs=NE))
            emb_pool = sd.enter_context(tc.tile_pool(name="embown", bufs=3))
            out_pool = sd.enter_context(tc.tile_pool(name="outsb", bufs=3))
            ops_pool = sd.enter_context(
                tc.tile_pool(name="op_ps", bufs=4, space="PSUM"))

            wot = []
            for e in range(NE):
                wo_t = wo_pool.tile([128, E], bf, name="wo_t")
                nc.sync.dma_start(wo_t[:], d["wo"][e * 128:(e + 1) * 128, :])
                wot.append(wo_t)

            for t in range(8):
                emb_t = emb_pool.tile([128, E], f32)
                nc.sync.dma_start(emb_t[:],
                                  d["emb_own"][t * 128:(t + 1) * 128, :])
                out_sb = out_pool.tile([128, E], f32)
                for j in range(4):
                    op = ops_pool.tile([128, 512], f32)
                    for kh in range(NE):
                        nc.tensor.matmul(
                            op[:], acat[kh][:, t * 128:(t + 1) * 128],
                            wot[kh][:, j * 512:(j + 1) * 512],
                            start=(kh == 0), stop=(kh == NE - 1))
                    nc.vector.tensor_add(out_sb[:, j * 512:(j + 1) * 512],
                                         op[:], emb_t[:, j * 512:(j + 1) * 512])
                nc.sync.dma_start(d["out"][t * 128:(t + 1) * 128, :],
                                  out_sb[:])
        acat_cm.__exit__(None, None, None)


_CACHED_NC = None


def build_graph():
    global _CACHED_NC
    if _CACHED_NC is not None:
        return _CACHED_NC
    nc = bacc.Bacc("TRN2", target_bir_lowering=False, debug=False,
                   enable_asserts=False, num_devices=8)
    d = {}
    d["embT"] = nc.dram_tensor("embT", [E, TALL], f32, kind="ExternalInput").ap()
    d["emb_own"] = nc.dram_tensor("emb_own", [TOWN, E], f32,
                                  kind="ExternalInput").ap()
    d["wq"] = nc.dram_tensor("wq", [H, E, DK], bf, kind="ExternalInput").ap()
    d["wk"] = nc.dram_tensor("wk", [E, HKV * DK], bf, kind="ExternalInput").ap()
    d["wv"] = nc.dram_tensor("wv", [E, HKV * DV], bf, kind="ExternalInput").ap()
    d["wo"] = nc.dram_tensor("wo", [H * DV, E], bf, kind="ExternalInput").ap()
    d["cosqT"] = nc.dram_tensor("cosqT", [DK, TOWN], bf, kind="ExternalInput").ap()
    d["sinqT"] = nc.dram_tensor("sinqT", [DK, TOWN], bf, kind="ExternalInput").ap()
    d["coskT"] = nc.dram_tensor("coskT", [DK, TALL], bf, kind="ExternalInput").ap()
    d["sinkT"] = nc.dram_tensor("sinkT", [DK, TALL], bf, kind="ExternalInput").ap()
    d["maskT"] = nc.dram_tensor("maskT", [NB, NQC, 128, NCH, 128], bf,
                                kind="ExternalInput").ap()
    d["out"] = nc.dram_tensor("out", [TOWN, E], f32, kind="ExternalOutput").ap()

    with tile.TileContext(nc, trace_sim=False) as tc:
        build(tc, d)
    nc.compile()
    _CACHED_NC = nc
    return nc


def make_in_maps(embeddings, cos_buffer, sin_buffer, wq, wk, wv, wo):
    embeddings = np.asarray(embeddings, dtype=np.float32)
    cos_buffer = np.asarray(cos_buffer, dtype=np.float32)
    sin_buffer = np.asarray(sin_buffer, dtype=np.float32)
    wq_s = (np.asarray(wq, np.float32) / math.sqrt(DK)).astype(BF16)
    wq_s = np.ascontiguousarray(wq_s.reshape(E, H, DK).transpose(1, 0, 2))
    wk_b = np.asarray(wk, np.float32).astype(BF16)
    wv_b = np.asarray(wv, np.float32).astype(BF16)
    wo_b = np.asarray(wo, np.float32).astype(BF16)

    in_maps = []
    for core in range(8):
        b, c = divmod(core, 4)
        tok0 = 1024 * c
        if c == 0:
            pad = np.zeros((HALO, E), np.float32)
            seg = np.concatenate([pad, embeddings[b, :TOWN]], axis=0)
            padc = np.zeros((HALO, DK), np.float32)
            ck = np.concatenate([padc, cos_buffer[1, 0, :TOWN]], axis=0)
            sk = np.concatenate([padc, sin_buffer[1, 0, :TOWN]], axis=0)
        else:
            seg = embeddings[b, tok0 - HALO:tok0 + TOWN]
            ck = cos_buffer[1, 0, tok0 - HALO:tok0 + TOWN]
            sk = sin_buffer[1, 0, tok0 - HALO:tok0 + TOWN]

        # masks [NB, NQC, 128(kk), NCH, 128(qq)] {0,1}
        mask = np.zeros((NB, NQC, 128, NCH, 128), np.float32)
        qq = np.arange(128)
        kk = np.arange(128)
        for blk in range(NB):
            for qc in range(NQC):
                qpos = tok0 + 512 * blk + 128 * qc + qq
                for ch in range(NCH):
                    kpos = tok0 - 512 + 512 * blk + 128 * qc + 128 * ch + kk
                    m = ((kpos[:, None] > qpos[None, :] - WIN)
                         & (kpos[:, None] <= qpos[None, :])
                         & (kpos[:, None] >= 0))
                    mask[blk, qc, :, ch, :] = m

        in_maps.append({
            "embT": np.ascontiguousarray(seg.T),
            "emb_own": np.ascontiguousarray(embeddings[b, tok0:tok0 + TOWN]),
            "wq": wq_s, "wk": wk_b, "wv": wv_b, "wo": wo_b,
            "cosqT": np.ascontiguousarray(
                cos_buffer[0, 0, tok0:tok0 + TOWN].T).astype(BF16),
            "sinqT": np.ascontiguousarray(
                sin_buffer[0, 0, tok0:tok0 + TOWN].T).astype(BF16),
            "coskT": np.ascontiguousarray(ck.T).astype(BF16),
            "sinkT": np.ascontiguousarray(sk.T).astype(BF16),
            "maskT": mask.astype(BF16),
        })
    return in_maps


def _install_ntff_hook():
    """Recreate the missing antenv.axon_hooks registry so
    run_bass_kernel_spmd(trace=True) can capture an NTFF profile."""
    import types
    if "antenv.axon_hooks" not in sys.modules:
        m = types.ModuleType("antenv.axon_hooks")
        m._hook = None
        m.set_axon_ntff_profile_hook = lambda h: setattr(m, "_hook", h)
        m.get_axon_ntff_profile_hook = lambda: m._hook
        sys.modules["antenv.axon_hooks"] = m
        try:
            import antenv
            antenv.axon_hooks = m
        except ImportError:
            pass
    try:
        from trn_agent_boot.trn_boot import _ntff_profile_via_ctypes
        hook = _ntff_profile_via_ctypes("/opt/axon/libaxon_pjrt.so")
        sys.modules["antenv.axon_hooks"].set_axon_ntff_profile_hook(hook)
    except Exception as exc:  # degrade to no tracing
        print(f"ntff hook install failed: {exc}", file=sys.stderr)


def kernel(embeddings, cos_buffer, sin_buffer, wq, wk, wv, wo, window_size,
           trace=False):
    assert int(window_size) == WIN
    if trace:
        _install_ntff_hook()
    nc = build_graph()
    in_maps = make_in_maps(embeddings, cos_buffer, sin_buffer, wq, wk, wv, wo)
    res = bass_utils.run_bass_kernel_spmd(
        nc, in_maps, core_ids=list(range(8)), trace=trace)
    out = np.zeros((B, S, E), np.float32)
    for core in range(8):
        b, c = divmod(core, 4)
        out[b, 1024 * c:1024 * (c + 1)] = res.results[core]["out"]
    if trace:
        kernel.last_exec_time_ns = res.exec_time_ns
    return out


kernel.last_exec_time_ns = None


# revision 14
# speedup vs baseline: 1.3784x; 1.0747x over previous
"""AthenaSA sliding-window attention layer on 8 TRN2 NeuronCores.

Sharding: sequence-parallel. 8 cores = 2 batches x 4 sequence chunks of 1024
tokens. Each core recomputes k/v for a 512-token halo (zero-padded for the
first chunk), so there are NO collectives — the kernel is embarrassingly
parallel and each core runs an identical Bass graph on different data.

Per-core pipeline (all matmuls bf16, accumulation f32):
  embT [E,1536] (host-pre-transposed) -> RMSNorm stats via ones-matmul
  partition reduction -> normedT bf16 -> QK projections in transposed layout
  (q^T, k^T = [dk, tokens]) + RoPE (partition-shifted reads) -> V projection
  in natural layout -> banded sliding-window attention with transposed
  scores (scores^T = k-chunk @ q^T), multiplicative data-driven masks after
  exp, denominators via all-ones stationary matmul (broadcast across
  partitions) -> out-projection back to natural layout + residual.
"""
import math
import os
import sys

sys.path.insert(0, "/opt/trn_rl_repo")

import numpy as np
import ml_dtypes

import concourse.bass as bass
import concourse.bacc as bacc
import concourse.mybir as mybir
from concourse import tile
from concourse import bass_utils
from contextlib import ExitStack

BF16 = ml_dtypes.bfloat16

B, S, E = 2, 4096, 2048
H, HKV, DK, DV = 16, 4, 128, 128
WIN = 512
EPS = 1e-5
TOWN, TALL, HALO = 1024, 1536, 512
NE = E // 128            # 16 e-tiles
NB = 2                   # window blocks per core
NQC = 4                  # query tiles of 128 per block
NCH = 5                  # key chunks of 128 per 640-window
GQ = H // HKV

f32 = mybir.dt.float32
bf = mybir.dt.bfloat16
AF = mybir.ActivationFunctionType


def build(tc, d):
    nc = tc.nc

    with ExitStack() as stage_all:
        stage_all.enter_context(
            nc.allow_low_precision(reason="bf16 compute path by design"))
        const_pool = stage_all.enter_context(tc.tile_pool(name="const", bufs=1))
        ones = const_pool.tile([128, 128], bf)
        nc.gpsimd.memset(ones[:], 1.0)
        epsb = const_pool.tile([128, 1], f32)
        nc.gpsimd.memset(epsb[:], EPS)

        # manually-scoped pools; LIFO open/close order:
        # acat -> kT -> v -> qT -> normedT
        acat_cm = tc.tile_pool(name="acat", bufs=H // 2)      # ..D
        acat_pool = acat_cm.__enter__()
        kT_cm = tc.tile_pool(name="kT", bufs=HKV)             # ..C
        kT_pool = kT_cm.__enter__()
        v_cm = tc.tile_pool(name="v", bufs=12)                # ..C
        v_pool = v_cm.__enter__()
        qT_cm = tc.tile_pool(name="qT", bufs=H // 2)          # ..C
        qT_pool = qT_cm.__enter__()
        normedT_cm = tc.tile_pool(name="normedT", bufs=NE)    # ..B2
        normedT_pool = normedT_cm.__enter__()

        # ---------------- Stage A: RMSNorm stats + normalize ----------------
        normedT = []
        with ExitStack() as sa:
            sq_pool = sa.enter_context(tc.tile_pool(name="sq", bufs=3))
            r_pool = sa.enter_context(tc.tile_pool(name="rms", bufs=1))
            ssq_psum = sa.enter_context(
                tc.tile_pool(name="ssq_ps", bufs=1, space="PSUM"))

            ssq = ssq_psum.tile([128, TALL], f32)  # 3 banks
            for e in range(NE):
                nt = normedT_pool.tile([128, TALL], bf, name="nt")
                nc.sync.dma_start(nt[:], d["embT"][e * 128:(e + 1) * 128, :])
                normedT.append(nt)
                sq = sq_pool.tile([128, TALL], bf)
                nc.vector.tensor_mul(sq[:], nt[:], nt[:])
                for j in range(3):
                    nc.tensor.matmul(
                        ssq[:, j * 512:(j + 1) * 512], ones[:],
                        sq[:, j * 512:(j + 1) * 512],
                        start=(e == 0), stop=(e == NE - 1))
            # rms = sqrt(ssq/E + eps); rb = 1/rms  (all rows identical)
            s_sb = r_pool.tile([128, TALL], f32)
            nc.scalar.activation(s_sb[:], ssq[:], AF.Sqrt,
                                 bias=epsb[:], scale=1.0 / E)
            rb = r_pool.tile([128, TALL], bf)
            nc.vector.reciprocal(rb[:], s_sb[:])             # f32 in, bf16 out
            for e in range(NE):
                nc.vector.tensor_mul(normedT[e][:], normedT[e][:], rb[:])

        # ---------------- Stage B1: K^T + V projections ----------------
        kT = []   # per kv head: [128(dk), TALL] bf16, rope'd
        vtiles = []  # 12 tiles [128(tok), 512(kv*dv)] bf16
        with ExitStack() as sb1:
            wk_pool = sb1.enter_context(tc.tile_pool(name="wk", bufs=NE))
            wv_pool = sb1.enter_context(tc.tile_pool(name="wv", bufs=NE))
            cs_pool = sb1.enter_context(tc.tile_pool(name="cosk", bufs=1))
            tmp_pool = sb1.enter_context(tc.tile_pool(name="ropetmp", bufs=3))
            kps_pool = sb1.enter_context(
                tc.tile_pool(name="k_ps", bufs=2, space="PSUM"))
            vps_pool = sb1.enter_context(
                tc.tile_pool(name="v_ps", bufs=2, space="PSUM"))

            wkt = []
            wvt = []
            for e in range(NE):
                wk_t = wk_pool.tile([128, HKV * DK], bf, name="wk_t")
                nc.sync.dma_start(wk_t[:], d["wk"][e * 128:(e + 1) * 128, :])
                wkt.append(wk_t)
                wv_t = wv_pool.tile([128, HKV * DV], bf, name="wv_t")
                nc.sync.dma_start(wv_t[:], d["wv"][e * 128:(e + 1) * 128, :])
                wvt.append(wv_t)
            cosk = cs_pool.tile([128, TALL], bf)
            sink = cs_pool.tile([128, TALL], bf)
            nc.sync.dma_start(cosk[:], d["coskT"][:, :])
            nc.sync.dma_start(sink[:], d["sinkT"][:, :])

            for hk in range(HKV):
                kps = kps_pool.tile([128, TALL], f32)  # 3 banks
                for e in range(NE):
                    for j in range(3):
                        nc.tensor.matmul(
                            kps[:, j * 512:(j + 1) * 512],
                            wkt[e][:, hk * DK:(hk + 1) * DK],
                            normedT[e][:, j * 512:(j + 1) * 512],
                            start=(e == 0), stop=(e == NE - 1))
                # rope: out = cos*kraw + sin*swap(kraw); swap rotates the two
                # 64-partition halves
                kraw = tmp_pool.tile([128, TALL], bf)
                nc.scalar.copy(kraw[:], kps[:])
                ksw = tmp_pool.tile([128, TALL], bf)
                nc.sync.dma_start(ksw[0:64, :], kraw[64:128, :])
                nc.sync.dma_start(ksw[64:128, :], kraw[0:64, :])
                t1 = tmp_pool.tile([128, TALL], bf)
                nc.vector.tensor_mul(t1[:], kraw[:], cosk[:])
                ko = kT_pool.tile([128, TALL], bf, name="ko")
                nc.vector.tensor_mul(ko[:], ksw[:], sink[:])
                nc.vector.tensor_add(ko[:], ko[:], t1[:])
                kT.append(ko)

            for t in range(12):
                vps = vps_pool.tile([128, HKV * DV], f32)  # 1 bank
                for e in range(NE):
                    nc.tensor.matmul(
                        vps[:], normedT[e][:, t * 128:(t + 1) * 128],
                        wvt[e][:], start=(e == 0), stop=(e == NE - 1))
                vt = v_pool.tile([128, HKV * DV], bf, name="vt")
                nc.scalar.copy(vt[:], vps[:])
                vtiles.append(vt)

        # ---------------- Stage B2: Q^T projection ----------------
        # wq comes in host-permuted per-head layout [H, E, DK] so per-head
        # weight tiles DMA contiguously and only ~3 heads stay resident.
        qT = []
        with ExitStack() as sb2:
            wq_pool = sb2.enter_context(tc.tile_pool(name="wq", bufs=3 * NE))
            csq_pool = sb2.enter_context(tc.tile_pool(name="cosq", bufs=1))
            tmpq_pool = sb2.enter_context(tc.tile_pool(name="ropetmpq", bufs=3))
            qps_pool = sb2.enter_context(
                tc.tile_pool(name="q_ps", bufs=3, space="PSUM"))

            cosq = csq_pool.tile([128, TOWN], bf)
            sinq = csq_pool.tile([128, TOWN], bf)
            nc.sync.dma_start(cosq[:], d["cosqT"][:, :])
            nc.sync.dma_start(sinq[:], d["sinqT"][:, :])

            for h in range(H):
                hh = h % 2
                if hh == 0:
                    qpair = qT_pool.tile([128, 2 * TOWN], bf, name="qpair")
                    qT.append(qpair)
                wqh = []
                for e in range(NE):
                    wq_t = wq_pool.tile([128, DK], bf, name="wq_t")
                    nc.sync.dma_start(wq_t[:],
                                      d["wq"][h, e * 128:(e + 1) * 128, :])
                    wqh.append(wq_t)
                qps = qps_pool.tile([128, TOWN], f32)  # 2 banks
                for e in range(NE):
                    for j in range(2):
                        nc.tensor.matmul(
                            qps[:, j * 512:(j + 1) * 512],
                            wqh[e][:],
                            normedT[e][:, HALO + j * 512:HALO + (j + 1) * 512],
                            start=(e == 0), stop=(e == NE - 1))
                qraw = tmpq_pool.tile([128, TOWN], bf)
                nc.scalar.copy(qraw[:], qps[:])
                qsw = tmpq_pool.tile([128, TOWN], bf)
                nc.sync.dma_start(qsw[0:64, :], qraw[64:128, :])
                nc.sync.dma_start(qsw[64:128, :], qraw[0:64, :])
                t1 = tmpq_pool.tile([128, TOWN], bf, name="t1q")
                nc.vector.tensor_mul(t1[:], qraw[:], cosq[:])
                # interleaved pair layout: columns (qtile, head, q)
                qo = qpair.rearrange("p (t g q) -> p t g q", g=2, q=128)[:, :, hh, :]
                nc.vector.tensor_mul(qo, qsw[:], sinq[:])
                nc.vector.tensor_add(qo, qo, t1[:])
        normedT_cm.__exit__(None, None, None)

        # ---------------- Stage C: attention ----------------
        acat = []
        for p in range(H // 2):
            acat.append(acat_pool.tile([128, 2 * TOWN], bf, name="acat"))

        with ExitStack() as sc_stage:
            mask_pool = sc_stage.enter_context(
                tc.tile_pool(name="mask", bufs=NB * NQC))
            probs_pool = sc_stage.enter_context(tc.tile_pool(name="probs", bufs=3))
            rec_pool = sc_stage.enter_context(tc.tile_pool(name="rec", bufs=3))
            scps_pool = sc_stage.enter_context(
                tc.tile_pool(name="sc_ps", bufs=2, space="PSUM"))
            dnps_pool = sc_stage.enter_context(
                tc.tile_pool(name="dn_ps", bufs=1, space="PSUM"))
            otps_pool = sc_stage.enter_context(
                tc.tile_pool(name="ot_ps", bufs=1, space="PSUM"))

            masks = {}
            for blk in range(NB):
                for qc in range(NQC):
                    m = mask_pool.tile([128, NCH * 256], bf, name="m")
                    nc.sync.dma_start(
                        m[:], d["maskT"][blk, qc].rearrange("k c g q -> k (c g q)"))
                    masks[(blk, qc)] = m

            for p in range(H // 2):
                kv = p // 2                        # 2 pairs per kv head
                for blk in range(NB):
                    for qc in range(NQC):
                        w0 = 512 * blk + 128 * qc     # key window start (local)
                        t = 4 * blk + qc              # own q-tile index
                        scp = scps_pool.tile([128, NCH * 256], f32)
                        for ch in range(NCH):
                            nc.tensor.matmul(
                                scp[:, ch * 256:(ch + 1) * 256],
                                kT[kv][:, w0 + ch * 128:w0 + (ch + 1) * 128],
                                qT[p][:, t * 256:(t + 1) * 256],
                                start=True, stop=True)
                        probs = probs_pool.tile([128, NCH * 256], bf)
                        nc.scalar.activation(probs[:], scp[:], AF.Exp)
                        nc.vector.tensor_mul(probs[:], probs[:],
                                             masks[(blk, qc)][:])
                        dn = dnps_pool.tile([128, 256], f32)
                        for ch in range(NCH):
                            nc.tensor.matmul(
                                dn[:], ones[:],
                                probs[:, ch * 256:(ch + 1) * 256],
                                start=(ch == 0), stop=(ch == NCH - 1))
                        rec = rec_pool.tile([128, 256], f32)
                        nc.vector.reciprocal_approx_fast(rec[:], dn[:])
                        otp = otps_pool.tile([128, 256], f32)
                        for ch in range(NCH):
                            vt = vtiles[4 * blk + qc + ch]
                            nc.tensor.matmul(
                                otp[:], vt[:, kv * DV:(kv + 1) * DV],
                                probs[:, ch * 256:(ch + 1) * 256],
                                start=(ch == 0), stop=(ch == NCH - 1))
                        nc.vector.tensor_mul(acat[p][:, t * 256:(t + 1) * 256],
                                             otp[:], rec[:])
        qT_cm.__exit__(None, None, None)
        v_cm.__exit__(None, None, None)
        kT_cm.__exit__(None, None, None)

        # ---------------- Stage D: out projection + residual ----------
        with ExitStack() as sd:
            wo_pool = sd.enter_context(tc.tile_pool(name="wo", bufs=NE))
            emb_pool = sd.enter_context(tc.tile_pool(name="embown", bufs=3))
            out_pool = sd.enter_context(tc.tile_pool(name="outsb", bufs=3))
            ops_pool = sd.enter_context(
                tc.tile_pool(name="op_ps", bufs=4, space="PSUM"))

            wot = []
            for e in range(NE):
                wo_t = wo_pool.tile([128, E], bf, name="wo_t")
                nc.sync.dma_start(wo_t[:], d["wo"][e * 128:(e + 1) * 128, :])
                wot.append(wo_t)

            for t in range(8):
                emb_t = emb_pool.tile([128, E], f32)
                nc.sync.dma_start(emb_t[:],
                                  d["emb_own"][t * 128:(t + 1) * 128, :])
                out_sb = out_pool.tile([128, E], f32)
                for j in range(4):
                    op = ops_pool.tile([128, 512], f32)
                    for kh in range(NE):
                        lhs = acat[kh // 2][:, (t * 2 + kh % 2) * 128:
                                            (t * 2 + kh % 2 + 1) * 128]
                        nc.tensor.matmul(
                            op[:], lhs,
                            wot[kh][:, j * 512:(j + 1) * 512],
                            start=(kh == 0), stop=(kh == NE - 1))
                    nc.vector.tensor_add(out_sb[:, j * 512:(j + 1) * 512],
                                         op[:], emb_t[:, j * 512:(j + 1) * 512])
                nc.sync.dma_start(d["out"][t * 128:(t + 1) * 128, :],
                                  out_sb[:])
        acat_cm.__exit__(None, None, None)


_CACHED_NC = None


def build_graph():
    global _CACHED_NC
    if _CACHED_NC is not None:
        return _CACHED_NC
    nc = bacc.Bacc("TRN2", target_bir_lowering=False, debug=False,
                   enable_asserts=False, num_devices=8)
    d = {}
    d["embT"] = nc.dram_tensor("embT", [E, TALL], bf, kind="ExternalInput").ap()
    d["emb_own"] = nc.dram_tensor("emb_own", [TOWN, E], f32,
                                  kind="ExternalInput").ap()
    d["wq"] = nc.dram_tensor("wq", [H, E, DK], bf, kind="ExternalInput").ap()
    d["wk"] = nc.dram_tensor("wk", [E, HKV * DK], bf, kind="ExternalInput").ap()
    d["wv"] = nc.dram_tensor("wv", [E, HKV * DV], bf, kind="ExternalInput").ap()
    d["wo"] = nc.dram_tensor("wo", [H * DV, E], bf, kind="ExternalInput").ap()
    d["cosqT"] = nc.dram_tensor("cosqT", [DK, TOWN], bf, kind="ExternalInput").ap()
    d["sinqT"] = nc.dram_tensor("sinqT", [DK, TOWN], bf, kind="ExternalInput").ap()
    d["coskT"] = nc.dram_tensor("coskT", [DK, TALL], bf, kind="ExternalInput").ap()
    d["sinkT"] = nc.dram_tensor("sinkT", [DK, TALL], bf, kind="ExternalInput").ap()
    d["maskT"] = nc.dram_tensor("maskT", [NB, NQC, 128, NCH, 2, 128], bf,
                                kind="ExternalInput").ap()
    d["out"] = nc.dram_tensor("out", [TOWN, E], f32, kind="ExternalOutput").ap()

    with tile.TileContext(nc, trace_sim=False) as tc:
        build(tc, d)
    nc.compile()
    _CACHED_NC = nc
    return nc


def make_in_maps(embeddings, cos_buffer, sin_buffer, wq, wk, wv, wo):
    embeddings = np.asarray(embeddings, dtype=np.float32)
    cos_buffer = np.asarray(cos_buffer, dtype=np.float32)
    sin_buffer = np.asarray(sin_buffer, dtype=np.float32)
    wq_s = (np.asarray(wq, np.float32) / math.sqrt(DK)).astype(BF16)
    wq_s = np.ascontiguousarray(wq_s.reshape(E, H, DK).transpose(1, 0, 2))
    wk_b = np.asarray(wk, np.float32).astype(BF16)
    wv_b = np.asarray(wv, np.float32).astype(BF16)
    wo_b = np.asarray(wo, np.float32).astype(BF16)

    in_maps = []
    for core in range(8):
        b, c = divmod(core, 4)
        tok0 = 1024 * c
        if c == 0:
            pad = np.zeros((HALO, E), np.float32)
            seg = np.concatenate([pad, embeddings[b, :TOWN]], axis=0)
            padc = np.zeros((HALO, DK), np.float32)
            ck = np.concatenate([padc, cos_buffer[1, 0, :TOWN]], axis=0)
            sk = np.concatenate([padc, sin_buffer[1, 0, :TOWN]], axis=0)
        else:
            seg = embeddings[b, tok0 - HALO:tok0 + TOWN]
            ck = cos_buffer[1, 0, tok0 - HALO:tok0 + TOWN]
            sk = sin_buffer[1, 0, tok0 - HALO:tok0 + TOWN]

        # masks [NB, NQC, 128(kk), NCH, 2(head), 128(qq)] {0,1}
        mask = np.zeros((NB, NQC, 128, NCH, 2, 128), np.float32)
        qq = np.arange(128)
        kk = np.arange(128)
        for blk in range(NB):
            for qc in range(NQC):
                qpos = tok0 + 512 * blk + 128 * qc + qq
                for ch in range(NCH):
                    kpos = tok0 - 512 + 512 * blk + 128 * qc + 128 * ch + kk
                    m = ((kpos[:, None] > qpos[None, :] - WIN)
                         & (kpos[:, None] <= qpos[None, :])
                         & (kpos[:, None] >= 0))
                    mask[blk, qc, :, ch, 0, :] = m
                    mask[blk, qc, :, ch, 1, :] = m

        in_maps.append({
            "embT": np.ascontiguousarray(seg.T).astype(BF16),
            "emb_own": np.ascontiguousarray(embeddings[b, tok0:tok0 + TOWN]),
            "wq": wq_s, "wk": wk_b, "wv": wv_b, "wo": wo_b,
            "cosqT": np.ascontiguousarray(
                cos_buffer[0, 0, tok0:tok0 + TOWN].T).astype(BF16),
            "sinqT": np.ascontiguousarray(
                sin_buffer[0, 0, tok0:tok0 + TOWN].T).astype(BF16),
            "coskT": np.ascontiguousarray(ck.T).astype(BF16),
            "sinkT": np.ascontiguousarray(sk.T).astype(BF16),
            "maskT": mask.astype(BF16),
        })
    return in_maps


def _install_ntff_hook():
    """Recreate the missing antenv.axon_hooks registry so
    run_bass_kernel_spmd(trace=True) can capture an NTFF profile."""
    import types
    if "antenv.axon_hooks" not in sys.modules:
        m = types.ModuleType("antenv.axon_hooks")
        m._hook = None
        m.set_axon_ntff_profile_hook = lambda h: setattr(m, "_hook", h)
        m.get_axon_ntff_profile_hook = lambda: m._hook
        sys.modules["antenv.axon_hooks"] = m
        try:
            import antenv
            antenv.axon_hooks = m
        except ImportError:
            pass
    try:
        from trn_agent_boot.trn_boot import _ntff_profile_via_ctypes
        hook = _ntff_profile_via_ctypes("/opt/axon/libaxon_pjrt.so")
        sys.modules["antenv.axon_hooks"].set_axon_ntff_profile_hook(hook)
    except Exception as exc:  # degrade to no tracing
        print(f"ntff hook install failed: {exc}", file=sys.stderr)


def kernel(embeddings, cos_buffer, sin_buffer, wq, wk, wv, wo, window_size,
           trace=False):
    assert int(window_size) == WIN
    if trace:
        _install_ntff_hook()
    nc = build_graph()
    in_maps = make_in_maps(embeddings, cos_buffer, sin_buffer, wq, wk, wv, wo)
    res = bass_utils.run_bass_kernel_spmd(
        nc, in_maps, core_ids=list(range(8)), trace=trace)
    out = np.zeros((B, S, E), np.float32)
    for core in range(8):
        b, c = divmod(core, 4)
        out[b, 1024 * c:1024 * (c + 1)] = res.results[core]["out"]
    if trace:
        kernel.last_exec_time_ns = res.exec_time_ns
    return out


kernel.last_exec_time_ns = None
